# revision 29
# baseline (speedup 1.0000x reference)
"""LowRankSparse2to4Linear Trainium2 kernel (v2).

out = (x16 @ A16) -> fp16 -> (@ B16^T) + bias, where A16/B16 are the 2:4
soft-thresholded (along rank), scaled, fp16-cast low-rank factors.

Strategy (8 NeuronCores, data-parallel over tokens, NO collectives):
  - tokens (8192) sharded 1024/core; every core receives the FULL weights
    and redundantly preprocesses them on-chip.
  - 2:4 soft-threshold in the NATURAL (interleaved) rank layout: one
    scalar-engine Abs activation casts f32->f16 while deinterleaving
    even/odd lanes (so the pair min/max tensor_tensor ops on DVE run
    unit-stride in the fp16 2x mode), then a fused SOFT_SHRINK custom
    DVE op reads the f32 weights directly with a broadcast threshold.
  - x is transposed on the tensor engine directly in f32 (transpose-mode
    matmuls); the PSUM->SBUF copy does the fp16 cast, so x never needs a
    separate cast pass.
  - GEMM1 computes x_projT = A_sp^T @ x^T; wave 1 is chunk-interleaved
    across 6 PSUM banks (both token halves share each stationary LDW) so
    the PE consumes weight chunks as DVE produces them, wave 2 finishes
    the remaining rank groups at full rate.
  - GEMM2 blocks are fed by JIT-preprocessed weight_B blocks that are
    transposed by the DMA X-bar (SBUF -> DRAM fp16 scratch -> one 3D
    transposed DMA read per half-block), keeping the PE free for pure
    GEMM work.  The first two blocks are prefetched before GEMM1 wave 2
    so the G1->G2 transition has its weights ready.
"""

import os
import sys
import numpy as np

sys.path.insert(0, "/opt/trn_rl_repo")

N_CORES = 8
IN_F, OUT_F, RANK = 4096, 4096, 1024
T_FULL = 8192             # 4 * 2048 tokens
TPC = T_FULL // N_CORES   # 1024 tokens per core

K_IN = IN_F // 128        # 32 contraction chunks for GEMM1
K_RK = RANK // 128        # 8 contraction chunks for GEMM2
NB = OUT_F // 512         # 8 output column blocks

_BUILD_CACHE = {}
_DVE_OPS = {}


def _register_custom_dve_ops():
    """Register the fused SOFT_SHRINK DVE op (runtime extension)."""
    if _DVE_OPS:
        return _DVE_OPS
    import numpy as _np
    from concourse import dve_ops
    from concourse.dve_spec import (Spec, Src0, Src1, Zero, minn, maxx,
                                    select, lower, _has_src1)
    from concourse.dve_uop import DveOpSpec

    def make_op(name, body, ref):
        existing = {op.name: op for op in dve_ops.OPS}
        if name in existing:
            return existing[name]
        spec = Spec(body=body, reference=ref)
        row = dve_ops._CUSTOM_DVE_ROW_BASE + len(dve_ops.OPS)
        shas = {}
        for ver in ("v3", "v4"):
            try:
                tmp = DveOpSpec(name=name, opcode=row, uops=lower(spec, ver=ver),
                                rd1_en=_has_src1(spec))
                shas[ver] = tmp.sha(ver)
            except Exception:
                pass
        op = dve_ops.DveOp(name, spec, subdim=False, uops_sha=shas)
        dve_ops.OPS.append(op)
        dve_ops.CUSTOM_DVE_SPECS[name] = spec
        dve_ops._SUB_OPCODE_FOR_NAME[name] = row
        return op

    _DVE_OPS["shrink"] = make_op(
        "SOFT_SHRINK_ANT",
        select(Src0 < Zero, minn(Src0 + Src1, Zero), maxx(Src0 - Src1, Zero)),
        lambda in0, in1, s0, s1, imm2: _np.where(
            in0 < 0, _np.minimum(in0 + in1, 0), _np.maximum(in0 - in1, 0)))
    return _DVE_OPS


def _build(scale_a: float, scale_b: float, bias_zero: bool):
    import concourse.bacc as bacc
    import concourse.tile as tile
    from concourse import mybir
    from concourse.masks import make_identity

    ops = _register_custom_dve_ops()

    f32 = mybir.dt.float32
    f16 = mybir.dt.float16
    Alu = mybir.AluOpType
    AF = mybir.ActivationFunctionType

    # engine for the pairwise min/max level of the threshold tree
    pair_eng_name = os.environ.get("LRS_PAIR_ENGINE", "vector")

    nc = bacc.Bacc("TRN2", target_bir_lowering=False, debug=False,
                   num_devices=N_CORES)

    x_sh = nc.dram_tensor("x_sh", [TPC, IN_F], f32, kind="ExternalInput")
    wa_d = nc.dram_tensor("wa_d", [IN_F, RANK], f32, kind="ExternalInput")
    wb_d = nc.dram_tensor("wb_d", [OUT_F, RANK], f32, kind="ExternalInput")
    bias_d = nc.dram_tensor("bias_d", [1, OUT_F], f32, kind="ExternalInput")
    out_d = nc.dram_tensor("out_d", [TPC, OUT_F], f32, kind="ExternalOutput")

    with tile.TileContext(nc) as tc:
        with (
            tc.tile_pool(name="singles", bufs=1) as singles,
            tc.tile_pool(name="wf32", bufs=3) as p_wf32,
            tc.tile_pool(name="mabs", bufs=2) as p_mabs,
            tc.tile_pool(name="mM", bufs=3) as p_mM,
            tc.tile_pool(name="eft", bufs=4) as p_eft,
            tc.tile_pool(name="wasp", bufs=32) as p_wasp,
            tc.tile_pool(name="wbsp", bufs=5) as p_wbsp,
            tc.tile_pool(name="xf32", bufs=2) as p_xf32,
            tc.tile_pool(name="xproj", bufs=16) as p_xp,
            tc.tile_pool(name="wbt", bufs=4) as p_wbt,
            tc.tile_pool(name="oev", bufs=3) as p_out,
            tc.tile_pool(name="bspd", bufs=3, space="DRAM") as p_bspd,
            tc.tile_pool(name="pstr", bufs=2, space="PSUM") as p_ps_tr,
            tc.tile_pool(name="psmm", bufs=6, space="PSUM") as p_ps_mm,
        ):
            ident32 = singles.tile([128, 128], f32)
            make_identity(nc, ident32[:])

            # resident transposed-x, one tensor per token-half:
            # [128 in-local, (in-chunk 32) x (tok 512)]
            xT = [singles.tile([128, K_IN * 512], f16, name=f"xT{th}")
                  for th in range(2)]

            def soft24_chunk(src_dram, row0, scale, dst_pool, name,
                             pair_eng, dma_eng=None, w_pool=None):
                """2:4 soft-threshold one (128, RANK) f32 row chunk into an
                fp16 tile in the NATURAL (interleaved) rank layout."""
                W = (w_pool or p_wf32).tile([128, RANK], f32, tag="wf",
                                            name=f"W_{name}")
                (dma_eng or nc.sync).dma_start(
                    W[:], src_dram[row0:row0 + 128, :])
                if scale != 1.0:
                    nc.scalar.mul(W[:], W[:], float(scale))
                # |W| cast to fp16, deinterleaved by 2: A2 = [ |w_even| | |w_odd| ]
                A2 = p_mabs.tile([128, RANK], f16, tag="mabs",
                                 name=f"A_{name}")
                nc.scalar.activation(
                    A2[:].rearrange("p (t q) -> p t q", t=2),
                    W[:].rearrange("p (q t) -> p t q", t=2),
                    AF.Abs)
                m = p_mM.tile([128, 512], f16, tag="mm", name=f"m_{name}")
                M = p_mM.tile([128, 512], f16, tag="mm", name=f"M_{name}")
                # unit-stride fp16 pair min/max (2x DVE mode)
                nc.vector.tensor_tensor(out=m[:], in0=A2[:, 0:512],
                                        in1=A2[:, 512:1024], op=Alu.min)
                nc.vector.tensor_tensor(out=M[:], in0=A2[:, 0:512],
                                        in1=A2[:, 512:1024], op=Alu.max)
                mv = m[:].rearrange("p (q t) -> p q t", t=2)  # [128,256,2]
                Mv = M[:].rearrange("p (q t) -> p q t", t=2)
                E = p_eft.tile([128, 256], f16, tag="eft", name=f"E_{name}")
                F = p_eft.tile([128, 256], f16, tag="eft", name=f"F_{name}")
                t = p_eft.tile([128, 256], f16, tag="eft", name=f"t_{name}")
                nc.vector.tensor_tensor(out=E[:], in0=mv[:, :, 0],
                                        in1=mv[:, :, 1], op=Alu.max)
                nc.vector.tensor_tensor(out=F[:], in0=Mv[:, :, 0],
                                        in1=Mv[:, :, 1], op=Alu.min)
                nc.vector.tensor_tensor(out=t[:], in0=E[:], in1=F[:],
                                        op=Alu.min)
                wsp = dst_pool.tile([128, RANK], f16, tag="wsp",
                                    name=f"wsp_{name}")
                nc.vector._custom_dve(
                    ops["shrink"],
                    out=wsp[:].rearrange("p (q f) -> p q f", f=4),
                    in0=W[:].rearrange("p (q f) -> p q f", f=4),
                    in1=t[:, :, None].to_broadcast([128, 256, 4]))
                return wsp

            def x_block(th, s, b):
                """DMA one [128 tok, 1024 in] f32 tile of x and transpose it
                (f32, tensor engine) into columns s*128.. of xT[th]."""
                tok0 = (th * 4 + s) * 128
                xf = p_xf32.tile([128, 1024], f32, tag="xf",
                                 name=f"xf_{th}_{s}_{b}")
                nc.sync.dma_start(
                    xf[:], x_sh[tok0:tok0 + 128, b * 1024:(b + 1) * 1024])
                xtv = xT[th][:].rearrange("p (i t) -> p i t", i=K_IN)
                for g in range(2):
                    pt = p_ps_tr.tile([128, 512], f32, tag="ps",
                                      name=f"pT_{th}_{s}_{b}_{g}")
                    for c in range(4):
                        icl = g * 4 + c
                        nc.tensor.transpose(
                            pt[:, c * 128:(c + 1) * 128],
                            xf[:, icl * 128:(icl + 1) * 128],
                            ident32[:])
                    ic0 = b * 8 + g * 4
                    dst = xtv[:, ic0:ic0 + 4, s * 128:(s + 1) * 128]
                    nc.scalar.copy(dst, pt[:])

            # ---- phase 1 emission: A-prep + x (both halves), DMA-interleaved
            wa_sp = [None] * K_IN
            xb0 = [(th, s, b) for b in range(4) for th in range(2)
                   for s in range(4)]
            for i in range(32):
                th, s, b = xb0[i]
                x_block(th, s, b)
                wa_sp[i] = soft24_chunk(wa_d, i * 128, scale_a, p_wasp,
                                        f"a{i}", pair_eng_name)

            # ---- bias broadcast (log-doubling), only if bias nonzero ----
            if not bias_zero:
                bias_bc = singles.tile([128, OUT_F], f32)
                nc.sync.dma_start(bias_bc[0:1, :], bias_d[:])
                k = 1
                while k < 128:
                    nc.sync.dma_start(bias_bc[k:2 * k, :], bias_bc[0:k, :])
                    k *= 2

            xproj = {}  # (th, rc) -> (128, 512) fp16 tile [rank-local, tok]

            def g1_drain(th, rc, acc, eng="scalar"):
                xp = p_xp.tile([128, 512], f16, tag="xp",
                               name=f"xp_{th}_{rc}")
                if eng == "scalar":
                    nc.scalar.copy(xp[:], acc[:])
                else:
                    nc.vector.tensor_copy(out=xp[:], in_=acc[:])
                xproj[(th, rc)] = xp

            # ---- GEMM1 wave 1: (rc 0-2) x (th 0-1) chunk-interleaved over
            # 6 banks; consecutive th-pairs share the stationary operand ----
            accs0 = {(rc, th): p_ps_mm.tile([128, 512], f32, tag="g1",
                                            name=f"g1w_{rc}_{th}")
                     for rc in range(3) for th in range(2)}
            for ic in range(K_IN):
                for rc in range(3):
                    for th in range(2):
                        nc.tensor.matmul(
                            accs0[(rc, th)][:],
                            wa_sp[ic][:, rc * 128:(rc + 1) * 128],
                            xT[th][:, ic * 512:(ic + 1) * 512],
                            start=(ic == 0), stop=(ic == K_IN - 1))
            # ---- weight_B JIT preprocessing + DMA-xbar transpose + GEMM2 ----
            def b_prep(nb):
                """Preprocess block nb of weight_B and DMA-transpose it into
                one [128 rank-local, (rc 8) x (out 512)] fp16 SBUF tile."""
                bd = p_bspd.tile([512, RANK], f16, tag="bd",
                                 name=f"bd_{nb}")
                for j in range(4):
                    ic = nb * 4 + j
                    wsp = soft24_chunk(wb_d, ic * 128, scale_b,
                                       p_wbsp, f"b{ic}", pair_eng_name,
                                       dma_eng=nc.scalar)
                    nc.sync.dma_start(bd[j * 128:(j + 1) * 128, :], wsp[:])
                wts = []
                for h in range(2):
                    wt = p_wbt.tile([128, 4 * 512], f16, tag="wbt",
                                    name=f"wbt_{nb}_{h}")
                    nc.sync.dma_start(
                        wt[:].rearrange("p (rc t) -> p rc t", rc=4),
                        bd[:, h * 512:(h + 1) * 512], transpose=True)
                    wts.append(wt)
                return wts

            # ---- prefetch first two weight_B blocks (emitted BEFORE the
            # wave-1 drains so their scalar/DVE work isn't stuck behind
            # drain instructions that wait on wave-1 matmul completion) ----
            for (rc, th), acc in accs0.items():
                g1_drain(th, rc, acc)

            wbt_q = [b_prep(0), b_prep(1)]

            # ---- GEMM1 wave 2: rc 3-7, group-major, th-paired ----
            for rc in range(3, K_RK):
                acc = {th: p_ps_mm.tile([128, 512], f32, tag="g1",
                                        name=f"g1_{rc}_{th}")
                       for th in range(2)}
                for ic in range(K_IN):
                    for th in range(2):
                        nc.tensor.matmul(
                            acc[th][:],
                            wa_sp[ic][:, rc * 128:(rc + 1) * 128],
                            xT[th][:, ic * 512:(ic + 1) * 512],
                            start=(ic == 0), stop=(ic == K_IN - 1))
                for th in range(2):
                    g1_drain(th, rc, acc[th])


            for nb in range(NB):
                wt = wbt_q.pop(0)
                for ts in range(TPC // 128):
                    th, tl = ts // 4, ts % 4
                    acc2 = p_ps_mm.tile([128, 512], f32, tag="g1",
                                        name=f"g2_{nb}_{ts}")
                    for rc in range(K_RK):
                        nc.tensor.matmul(
                            acc2[:],
                            xproj[(th, rc)][:, tl * 128:(tl + 1) * 128],
                            wt[rc // 4][:, (rc % 4) * 512:
                                         (rc % 4 + 1) * 512],
                            start=(rc == 0), stop=(rc == K_RK - 1))
                    ot = p_out.tile([128, 512], f32, tag="oev",
                                    name=f"ot_{nb}_{ts}")
                    if bias_zero:
                        if nb >= NB - 2 and ts % 2 == 1:
                            nc.scalar.copy(ot[:], acc2[:])
                        else:
                            nc.vector.tensor_copy(out=ot[:], in_=acc2[:])
                    else:
                        nc.vector.tensor_tensor(
                            out=ot[:], in0=acc2[:],
                            in1=bias_bc[:, nb * 512:(nb + 1) * 512],
                            op=Alu.add)
                    nc.sync.dma_start(
                        out_d[ts * 128:(ts + 1) * 128,
                              nb * 512:(nb + 1) * 512],
                        ot[:])
                if nb + 2 < NB:
                    wbt_q.append(b_prep(nb + 2))

    nc.compile()
    return nc


def kernel(x, weight_A, weight_B, bias, scale_A, scale_B):
    from concourse.bass_utils import run_bass_kernel_spmd

    x = np.ascontiguousarray(np.asarray(x, dtype=np.float32))
    weight_A = np.ascontiguousarray(np.asarray(weight_A, dtype=np.float32))
    weight_B = np.ascontiguousarray(np.asarray(weight_B, dtype=np.float32))
    bias = np.ascontiguousarray(np.asarray(bias, dtype=np.float32))
    sa = float(np.asarray(scale_A))
    sb = float(np.asarray(scale_B))
    bias_zero = bool(np.all(bias == 0.0))

    lead = x.shape[:-1]
    xf = x.reshape(-1, IN_F)
    assert xf.shape == (T_FULL, IN_F)

    key = (sa, sb, bias_zero, os.environ.get("LRS_PAIR_ENGINE", "vector"))
    if key not in _BUILD_CACHE:
        _BUILD_CACHE[key] = _build(sa, sb, bias_zero)
    nc = _BUILD_CACHE[key]

    bias_row = bias.reshape(1, OUT_F)
    in_maps = []
    for c in range(N_CORES):
        in_maps.append({
            "x_sh": xf[c * TPC:(c + 1) * TPC],
            "wa_d": weight_A,
            "wb_d": weight_B,
            "bias_d": bias_row,
        })

    trace = os.environ.get("BASS_KERNEL_TRACE", "0") == "1"
    kwargs = {}
    if trace:
        _install_ntff_hook()
        kwargs["trace"] = True
        tmpdir = os.environ.get("BASS_KERNEL_TRACE_DIR")
        if tmpdir:
            os.makedirs(tmpdir, exist_ok=True)
            kwargs["tmpdir"] = tmpdir

    res = run_bass_kernel_spmd(nc, in_maps, core_ids=list(range(N_CORES)),
                               **kwargs)
    if trace:
        kernel.last_exec_time_ns = res.exec_time_ns

    out = np.empty((T_FULL, OUT_F), dtype=np.float32)
    for c in range(N_CORES):
        out[c * TPC:(c + 1) * TPC] = res.results[c]["out_d"]
    return out.reshape(*lead, OUT_F)


def _install_ntff_hook():
    """Provide antenv.axon_hooks (missing in this image) so trace=True works."""
    import types
    if "antenv.axon_hooks" in sys.modules:
        return
    try:
        from trn_agent_boot.trn_boot import _ntff_profile_via_ctypes
        hook = _ntff_profile_via_ctypes("/opt/axon/libaxon_pjrt.so")
    except Exception:
        hook = None
    mod = types.ModuleType("antenv.axon_hooks")
    mod.get_axon_ntff_profile_hook = lambda: hook
    mod.set_axon_ntff_profile_hook = lambda h: None
    import antenv  # noqa: F401
    sys.modules["antenv.axon_hooks"] = mod
